# revision 31
# baseline (speedup 1.0000x reference)
"""Bass/Tile TRN2 kernel for nn_BigramLanguageModel (8-layer dense transformer).

Data-parallel over batch across 8 NeuronCores (8 items/core, no collectives).
Residual stream kept feature-major ([C, tokens]) in SBUF so every matmul
contracts over the partition dim without transposes.

v2 restructure (vs v1): the per-head attention chain is PE->Act->PE only —
the causal mask is folded into the scores PSUM group as small constant
matmuls (-30 triangles; exp(-30) flushes to 0 in fp16), so each head needs
ONE exp activation and no GPSIMD/DVE mask op. Heads are emitted with a
sliding window (scores run 2 heads ahead of o/den) so PE never waits on the
Act exp round-trip. LN chains are software-pipelined: LN1 of chunk ti+1 is
computed (stats/finish/bcast during the LN2 DVE latency of chunk ti, apply
during FFN) so PE's dense stream is uninterrupted. Weights double-buffered
across layers.
"""

import os
import sys
from contextlib import ExitStack

import numpy as np

for _p in ("/opt/trn_rl_repo", "/root/.axon_site/_ro/trn_rl_repo"):
    if os.path.isdir(_p) and _p not in sys.path:
        sys.path.insert(0, _p)
        break

import concourse.bass as bass
import concourse.mybir as mybir
import concourse.tile as tile
from concourse import bacc

# model config (hardcoded per problem spec)
B, T, C, H, L, V = 64, 256, 512, 8, 8, 100
HD = C // H          # 64
FF = 4 * C           # 2048
EPS = 1e-5
NCORES = 8
BL = B // NCORES     # 8 batch items per core
NT = BL * T          # 2048 tokens per core
P = 128
NCC = C // P         # 4 c-chunks
NFF = FF // P        # 16 ff-chunks
TCH = 512            # token chunk (2 batch items)
NTC = NT // TCH      # 4
BI = TCH // T        # 2 batch items per token chunk

QS8 = 2048.0   # fp8 pre-scale for folded Wq (tiny: has 1/sqrt(C) baked in)
KS8 = 64.0     # fp8 pre-scale for folded Wk
F32 = mybir.dt.float32
F16 = mybir.dt.float16
F32R = mybir.dt.float32r
ADD = mybir.AluOpType.add
MULT = mybir.AluOpType.mult
AF = mybir.ActivationFunctionType


def _r(ap):
    return ap if ap.dtype == F32R else ap.bitcast(F32R)


def _bcast_dram(vec_ap, parts):
    return bass.AP(
        tensor=vec_ap.tensor,
        offset=vec_ap.offset,
        ap=[[0, parts]] + [list(d) for d in vec_ap.ap],
    )


PHASE_MARKS = []


def build_bass(zero_attn_bias=False, zero_mlp_bias=False):
    nc = bacc.Bacc()
    PHASE_MARKS.clear()

    def mark(label):
        PHASE_MARKS.append(
            (label, int(nc.get_next_instruction_name().split("-")[1])))
    dp = nc.declare_dram_parameter

    onehot_d = dp("onehotT", [V, NT], F16, False)
    tok_d = dp("tok_emb16", [V, C], F16, False)
    pos16_d = dp("pos2T", [C, TCH], F16, False)
    i16_d = dp("i16", [P, P], F16, False)        # identity
    mall_d = dp("mall", [P, 2, T], F16, False)   # additive causal mask, -30s
    wq8_d = dp("wq8", [L, 2, P, 2, C], mybir.dt.float8e4, False)
    wk8_d = dp("wk8", [L, 2, P, 2, C], mybir.dt.float8e4, False)
    wv_d = dp("wv", [L, C, C], F16, False)
    wo_d = dp("wo", [L, C, C], F16, False)
    w1_d = dp("w1", [L, C, FF], F16, False)
    w2_d = dp("w2", [L, FF, C], F16, False)
    bq_d = dp("bq", [L, C], F32, False)
    bk_d = dp("bk", [L, C], F32, False)
    bv_d = dp("bv", [L, C], F32, False)
    bo_d = dp("bo", [L, C], F32, False)
    b1_d = dp("b1", [L, FF], F32, False)
    b2_d = dp("b2", [L, C], F32, False)
    e8_d = dp("e8sel", [NCC, H, P], F32, False)
    wlm_d = dp("wlm", [C, V], F32, False)
    blm_d = dp("blm", [V], F32, False)
    out_d = dp("out", [NT, V], F32, True)

    with tile.TileContext(nc) as tc, ExitStack() as ctx:
        # ---------------- pools ----------------
        pconst = ctx.enter_context(tc.tile_pool(name="const", bufs=1))
        px = ctx.enter_context(tc.tile_pool(name="x", bufs=1))
        pw = ctx.enter_context(tc.tile_pool(name="w", bufs=1))
        pwq = ctx.enter_context(tc.tile_pool(name="wq", bufs=2))
        pbias = ctx.enter_context(tc.tile_pool(name="bias", bufs=2))
        ph = ctx.enter_context(tc.tile_pool(name="h", bufs=1))
        pq = ctx.enter_context(tc.tile_pool(name="q", bufs=1))
        pv = ctx.enter_context(tc.tile_pool(name="v", bufs=1))
        po = ctx.enter_context(tc.tile_pool(name="o", bufs=1))
        pffn = ctx.enter_context(tc.tile_pool(name="ffn", bufs=1))
        psq = ctx.enter_context(tc.tile_pool(name="sq", bufs=2))
        pstat = ctx.enter_context(tc.tile_pool(name="stat", bufs=2))
        pln = ctx.enter_context(tc.tile_pool(name="ln", bufs=1))
        pe_ = ctx.enter_context(tc.tile_pool(name="e", bufs=6))
        prd = ctx.enter_context(tc.tile_pool(name="rd", bufs=2))
        plog = ctx.enter_context(tc.tile_pool(name="log", bufs=2))
        # PSUM: 8 banks total
        pmm = ctx.enter_context(tc.tile_pool(name="mm", bufs=2, space="PSUM"))
        psc = ctx.enter_context(tc.tile_pool(name="scps", bufs=3, space="PSUM"))
        pops = ctx.enter_context(tc.tile_pool(name="ops", bufs=2, space="PSUM"))
        pden = ctx.enter_context(tc.tile_pool(name="den", bufs=1, space="PSUM"))

        # ---------------- constants ----------------
        # ones vector pre-scaled by 1/C so LN stats matmuls emit means
        ones_f = pconst.tile([P, 1], F32, tag="ones_f", name="ones_f")
        nc.vector.memset(ones_f, 1.0 / C)
        ones = pconst.tile([P, 1], F32R, tag="ones", name="ones")
        nc.vector.tensor_copy(ones, ones_f)
        ones16 = pconst.tile([P, 1], F16, tag="ones16", name="ones16")
        nc.vector.tensor_copy(ones16, ones_f)
        ones1_f = pconst.tile([1, P], F32, tag="ones1_f", name="ones1_f")
        nc.vector.memset(ones1_f, 1.0)
        ones1 = pconst.tile([1, P], F32R, tag="ones1", name="ones1")
        nc.vector.tensor_copy(ones1, ones1_f)
        eps_t = pconst.tile([1, 1], F32, tag="eps", name="eps")
        nc.vector.memset(eps_t, EPS)
        warm_t = pconst.tile([1, 1], F32, tag="warm", name="warm")
        i16 = pconst.tile([P, P], F16, tag="i16", name="i16")
        nc.sync.dma_start(out=i16, in_=i16_d[:, :])
        mall = pconst.tile([P, 2, T], F16, tag="mall", name="mall")
        nc.sync.dma_start(out=mall, in_=mall_d[:, :, :])
        tok_sb = pconst.tile([V, C], F16, tag="tok", name="tok")
        nc.sync.dma_start(out=tok_sb, in_=tok_d[:, :])
        wlm_sb = []
        for cc in range(NCC):
            f = pconst.tile([P, V], F32, tag=f"wlmf{cc}", name=f"wlmf{cc}")
            nc.sync.dma_start(out=f, in_=wlm_d[cc * P:(cc + 1) * P, :])
            t = pconst.tile([P, V], F32R, tag=f"wlm{cc}", name=f"wlm{cc}")
            nc.vector.tensor_copy(t, f)
            wlm_sb.append(t)
        blm_bc = pconst.tile([P, V], F32, tag="blm", name="blm")
        nc.sync.dma_start(out=blm_bc, in_=_bcast_dram(blm_d[:], P))
        e8 = []
        for hq in range(NCC):
            f = pconst.tile([H, P], F32, tag=f"e8f{hq}", name=f"e8f{hq}")
            nc.sync.dma_start(out=f, in_=e8_d[hq])
            r8 = pconst.tile([H, P], F32R, tag=f"e8{hq}", name=f"e8{hq}")
            nc.vector.tensor_copy(r8, f)
            e8.append(r8)
        # one-hot columns for denominator matmuls: o8c[p, hh, j] = (j == hh)
        o8c = pconst.tile([P, H, H], F16, tag="o8c", name="o8c")
        nc.vector.memset(o8c, 0.0)
        for hh in range(H):
            nc.vector.memset(o8c[:, hh, hh:hh + 1], 1.0)

        # resident residual stream, feature-major: x_T[c, t]
        x_sb = [px.tile([P, NT], F32R, tag=f"x{cc}", name=f"x{cc}")
                for cc in range(NCC)]

        # ---------------- embedding ----------------
        with tc.tile_pool(name="emb", bufs=1) as pemb:
            oh_sb = pemb.tile([V, NT], F16, tag="oh", name="oh")
            nc.sync.dma_start(out=oh_sb, in_=onehot_d[:, :])
            pos_sb = []
            for cc in range(NCC):
                t = pemb.tile([P, TCH], F16, tag=f"pos{cc}", name=f"pos{cc}")
                nc.sync.dma_start(out=t, in_=pos16_d[cc * P:(cc + 1) * P, :])
                pos_sb.append(t)
            for ti in range(NTC):
                tsl = slice(ti * TCH, (ti + 1) * TCH)
                for cc in range(NCC):
                    ps = pmm.tile([P, TCH], F32, tag="mm", name="mmps")
                    nc.tensor.matmul(ps, tok_sb[:, cc * P:(cc + 1) * P],
                                     oh_sb[:, tsl], start=True, stop=True)
                    nc.vector.tensor_add(x_sb[cc][:, tsl], ps, pos_sb[cc])

        # ---------------- LN building blocks ----------------
        def ln_stats(tsl, pool=None):
            """S0 = mean(x), S1 = mean(x^2) via 1/C-ones matmuls on PE.
            squares computed on the Act engine (fp16) to keep DVE clear."""
            pool = pool or psc
            S0 = pool.tile([1, TCH], F32, tag=pool is psc and "sc" or "mm",
                           name="S0")
            S1 = pool.tile([1, TCH], F32, tag=pool is psc and "sc" or "mm",
                           name="S1")
            for cc in range(NCC):
                sq = psq.tile([P, TCH], F16, tag="sq", name="sq")
                nc.scalar.square(sq, x_sb[cc][:, tsl].bitcast(F32))
                nc.tensor.matmul(S0[0:1, :], _r(ones[:, :]), x_sb[cc][:, tsl],
                                 start=(cc == 0), stop=(cc == NCC - 1))
                nc.tensor.matmul(S1[0:1, :], ones16[:, :], sq[:, :],
                                 start=(cc == 0), stop=(cc == NCC - 1))
            return S0, S1

        def ln_stats_bi(tsl, S0, S1, bi):
            """half-chunk stats: each bi's groups open their own region
            (replace-first on every byte under either PSUM-start model)."""
            bsl = slice(bi * T, (bi + 1) * T)
            xsl = slice(tsl.start + bi * T, tsl.start + (bi + 1) * T)
            for cc in range(NCC):
                sq = psq.tile([P, T], F16, tag="sq", name="sq")
                nc.scalar.square(sq, x_sb[cc][:, xsl].bitcast(F32))
                nc.tensor.matmul(S0[0:1, bsl], _r(ones[:, :]),
                                 x_sb[cc][:, xsl], start=(cc == 0),
                                 stop=(cc == NCC - 1),
                                 skip_group_check=True)
                nc.tensor.matmul(S1[0:1, bsl], ones16[:, :], sq[:, :],
                                 start=(cc == 0), stop=(cc == NCC - 1),
                                 skip_group_check=True)

        def ln_finish_half(S0, S1, m_t, m2_t, v_t, b_t, bsl):
            """finish on one half of the stats row (overlaps the other bi)."""
            nc.vector.tensor_copy(m_t[:, bsl], S0[0:1, bsl])
            nc.vector.tensor_mul(m2_t[:, bsl], m_t[:, bsl], m_t[:, bsl])
            nc.vector.tensor_sub(v_t[:, bsl], S1[0:1, bsl], m2_t[:, bsl])
            nc.scalar.activation(v_t[:, bsl], v_t[:, bsl], AF.Sqrt,
                                 bias=eps_t[:, :], scale=1.0)
            with nc.allow_low_precision("fp32r rstd is fp32-equivalent"):
                nc.vector.reciprocal(v_t[:, bsl], v_t[:, bsl])
            nc.vector.scalar_tensor_tensor(b_t[:, bsl], m_t[:, bsl], -1.0,
                                           v_t[:, bsl], op0=MULT, op1=MULT)

        def ln_alloc_fin():
            m_t = pstat.tile([1, TCH], F32R, tag="m0", name="m_t")
            m2_t = pstat.tile([1, TCH], F32, tag="m2", name="m2_t")
            v_t = pstat.tile([1, TCH], F32R, tag="v", name="v_t")
            b_t = pstat.tile([1, TCH], F32R, tag="m", name="b_t")
            return m_t, m2_t, v_t, b_t

        def ln_finish(S0, S1):
            """A = rstd, B = -mean*rstd (S0 = mean, S1 = E[x^2] already).
            sqrt table set load is prepaid by a warmup op late in the
            attention phase."""
            m_t = pstat.tile([1, TCH], F32R, tag="m0", name="m_t")
            m2_t = pstat.tile([1, TCH], F32, tag="m2", name="m2_t")
            v_t = pstat.tile([1, TCH], F32R, tag="v", name="v_t")
            b_t = pstat.tile([1, TCH], F32R, tag="m", name="b_t")
            nc.vector.tensor_copy(m_t, S0[0:1, :])
            nc.vector.tensor_mul(m2_t, m_t, m_t)
            nc.vector.tensor_sub(v_t, S1[0:1, :], m2_t)
            nc.scalar.activation(v_t, v_t, AF.Sqrt, bias=eps_t[:, :], scale=1.0)
            with nc.allow_low_precision("fp32r rstd is fp32-equivalent"):
                nc.vector.reciprocal(v_t, v_t)
            nc.vector.scalar_tensor_tensor(b_t, m_t, -1.0, v_t,
                                           op0=MULT, op1=MULT)
            return v_t, b_t

        def ln_bcast(v_t, m_t, atag, evac=False):
            """broadcast A,B rows to [P, TCH] PSUM; LN2 applies straight from
            PSUM (critical path), LN1-next evacuates to SBUF off-path so the
            psc ring is free during FFN"""
            a_ps = psc.tile([P, TCH], F32, tag="sc", name="a_ps")
            nc.tensor.matmul(a_ps, _r(ones1[:, :]), v_t[:, :],
                             start=True, stop=True)
            b_ps = psc.tile([P, TCH], F32, tag="sc", name="b_ps")
            nc.tensor.matmul(b_ps, _r(ones1[:, :]), m_t[:, :],
                             start=True, stop=True)
            if not evac:
                return a_ps, b_ps
            a_sb = pln.tile([P, TCH], F32, tag=f"{atag}a", name=f"{atag}a")
            b_sb = pln.tile([P, TCH], F32, tag=f"{atag}b", name=f"{atag}b")
            nc.vector.tensor_copy(a_sb, a_ps)
            nc.vector.tensor_copy(b_sb, b_ps)
            return a_sb, b_sb

        def ln_apply(tsl, a_sb, b_sb, htag):
            h = []
            h8 = [ph.tile([P, 2, TCH], mybir.dt.float8e4, tag=f"{htag}8{kp}",
                          name=f"h8{kp}") for kp in range(2)] \
                if htag == "h" else None
            for cc in range(NCC):
                d = ph.tile([P, TCH], F16, tag=f"{htag}{cc}", name=f"h{cc}")
                nc.vector.tensor_mul(d, x_sb[cc][:, tsl], a_sb)
                nc.vector.tensor_add(d, d, b_sb)
                if h8 is not None:
                    nc.scalar.copy(h8[cc // 2][:, cc % 2, :], d)
                h.append(d)
            if h8 is not None:
                return h, h8
            return h

        # ---------------- per-layer weights ----------------
        def load_weights(l):
            def _load(dram, tag, n, width, pool=pw):
                ts_ = []
                for i in range(n):
                    t = pool.tile([P, width], F16, tag=f"{tag}{i}", name=f"{tag}{i}")
                    nc.sync.dma_start(out=t, in_=dram[l, i * P:(i + 1) * P, :])
                    ts_.append(t)
                return ts_

            w = {}
            # qkv double-buffered (needed right at layer start); the rest
            # single-buffered (ample DMA slack before first use)
            F8 = mybir.dt.float8e4
            for nm, dr in (("wq8", wq8_d), ("wk8", wk8_d)):
                ts_ = []
                for kp in range(2):
                    t = pwq.tile([P, 2, C], F8, tag=f"{nm}{kp}",
                                 name=f"{nm}{kp}")
                    nc.sync.dma_start(out=t, in_=dr[l, kp])
                    ts_.append(t)
                w[nm] = ts_
            w["wv"] = _load(wv_d, "wv", NCC, C, pwq)
            w["wo"] = _load(wo_d, "wo", NCC, C)
            w["w1"] = _load(w1_d, "w1", NCC, FF)
            w["w2"] = _load(w2_d, "w2", NFF, C)
            if not zero_attn_bias:
                for nm, dr in (("bq", bq_d), ("bk", bk_d), ("bo", bo_d)):
                    t = pbias.tile([P, NCC], F32, tag=nm, name=nm)
                    nc.sync.dma_start(out=t, in_=dr[l].rearrange("(a p) -> p a", p=P))
                    w[nm] = t
                bv_bc = pbias.tile([P, C], F32, tag="bvb", name="bvb")
                nc.sync.dma_start(out=bv_bc, in_=_bcast_dram(bv_d[l], P))
                w["bv_bc"] = bv_bc
            if not zero_mlp_bias:
                t = pbias.tile([P, NFF], F32, tag="b1", name="b1")
                nc.sync.dma_start(out=t, in_=b1_d[l].rearrange("(a p) -> p a", p=P))
                w["b1"] = t
                t = pbias.tile([P, NCC], F32, tag="b2", name="b2")
                nc.sync.dma_start(out=t, in_=b2_d[l].rearrange("(a p) -> p a", p=P))
                w["b2"] = t
            return w

        # ---------------- attention per (bi, head) ----------------
        def attn_scores(q_t, k_t, bi, hh):
            """scores + mask into one PSUM bank; returns (sc_ps,)"""
            hq, hr = divmod(hh, 2)
            rsl = slice(hr * HD, (hr + 1) * HD)
            qsl = q_t[hq][rsl, bi * T:(bi + 1) * T]
            ksl0 = k_t[hq][rsl, bi * T: bi * T + P]
            ksl1 = k_t[hq][rsl, bi * T + P: bi * T + 2 * P]
            sc = psc.tile([P, 2, T], F32, tag="sc", name="sc")
            # causal mask folded in as additive -30 constants; ordering keeps
            # replace-then-accumulate consistent on every byte: s0 scores open
            # [.,0,.], the diag triangle accumulates, the s1 mask block opens
            # [.,1,.], s1 scores accumulate.
            nc.tensor.matmul(sc[:, 0, :], ksl0, qsl, start=True, stop=False)
            nc.tensor.matmul(sc[:, 0, 0:P], i16[:, :], mall[:, 0, 0:P],
                             start=False, stop=False)
            nc.tensor.matmul(sc[:, 1, :], i16[:, :], mall[:, 1, :],
                             start=True, stop=False, skip_group_check=True)
            nc.tensor.matmul(sc[:, 1, P:T], ksl1, qsl[:, P:T],
                             start=False, stop=True)
            e = pe_.tile([P, 2, T], F16, tag="e", name="e")
            nc.scalar.activation(e[:, :, :], sc[:, :, :], AF.Exp)
            return e

        def attn_odens(v8, e, o_t, den_ps, bi, hh):
            hq, hr = divmod(hh, 2)
            rsl = slice(hr * HD, (hr + 1) * HD)
            o_ps = pops.tile([HD, T], F32, tag="o", name="o_ps")
            nc.tensor.matmul(o_ps, v8[bi * 2][:, hh, :], e[:, 0, :],
                             start=True, stop=False)
            nc.tensor.matmul(o_ps[:, P:T], v8[bi * 2 + 1][:, hh, :],
                             e[:, 1, P:T], start=False, stop=True)
            nc.tensor.matmul(den_ps, o8c[:, hh, :], e[:, 0, :],
                             start=(hh == 0), stop=False)
            nc.tensor.matmul(den_ps[:, P:T], o8c[:, hh, :], e[:, 1, P:T],
                             start=False, stop=(hh == H - 1))
            nc.scalar.copy(o_t[hq][rsl, bi * T:(bi + 1) * T], o_ps[0:HD, :])

        def attn_recip(den_ps):
            rden = prd.tile([H, T], F32R, tag="rden", name="rden")
            with nc.allow_low_precision("fp32r rden is fp32-equivalent"):
                nc.vector.reciprocal(rden, den_ps)
            return rden

        def attn_rdbmul(o_t, rden, bi):
            for hq in range(NCC):
                rdb = pmm.tile([P, T], F32, tag="mm", name="rdb")
                nc.tensor.matmul(rdb, e8[hq][:, :], rden[:, :],
                                 start=True, stop=True)
                osl = o_t[hq][:, bi * T:(bi + 1) * T]
                nc.vector.tensor_mul(osl, osl, rdb)

        def proj_bi(wt, o_t, tsl, bi):
            """proj for one batch item; accumulate into x + LN2 stats gate"""
            bsl = slice(bi * T, (bi + 1) * T)
            xsl = slice(tsl.start + bi * T, tsl.start + (bi + 1) * T)
            for cc in range(NCC):
                pool = pmm if cc % 2 == 0 else psc
                ps = pool.tile([P, T], F32,
                               tag="mm" if pool is pmm else "sc",
                               name="prps")
                for hq in range(NCC):
                    nc.tensor.matmul(ps, wt["wo"][hq][:, cc * P:(cc + 1) * P],
                                     o_t[hq][:, bsl], start=(hq == 0),
                                     stop=(hq == NCC - 1))
                if zero_attn_bias:
                    nc.vector.tensor_add(x_sb[cc][:, xsl], ps,
                                         x_sb[cc][:, xsl])
                else:
                    nc.vector.scalar_tensor_tensor(
                        x_sb[cc][:, xsl], ps, wt["bo"][:, cc:cc + 1],
                        x_sb[cc][:, xsl], op0=ADD, op1=ADD)

        # ---------------- qkv ----------------
        def qkv_qk(wt, h8):
            """q,k via fp8e4 DoubleRow (K=256 per instruction); the weight
            pre-scales (QS8/KS8) are undone in the PSUM evacuation"""
            DRM = mybir.MatmulPerfMode.DoubleRow
            q_t, k_t = [], []
            gi = 0
            for dst, wnm, bnm, usc in ((q_t, "wq8", "bq", 1.0 / QS8),
                                       (k_t, "wk8", "bk", 1.0 / KS8)):
                for hq in range(NCC):
                    # alternate PSUM pools: short DR groups would otherwise
                    # serialize on the 2-bank ring's evacuation latency
                    pool = pmm if gi % 2 == 0 else pops
                    gi += 1
                    ps = pool.tile([P, TCH], F32,
                                   tag="mm" if pool is pmm else "o",
                                   name="mmps")
                    for kp in range(2):
                        nc.tensor.matmul(
                            ps, wt[wnm][kp][:, :, hq * P:(hq + 1) * P],
                            h8[kp][:, :, :], start=(kp == 0),
                            stop=(kp == 1), perf_mode=DRM)
                    qt = pq.tile([P, TCH], F16, tag=f"{bnm}t{hq}",
                                 name=f"{bnm}t{hq}")
                    if zero_attn_bias:
                        nc.scalar.mul(qt, ps, usc)
                    else:
                        nc.scalar.activation(qt, ps, AF.Identity,
                                             bias=wt[bnm][:, hq:hq + 1],
                                             scale=usc)
                    dst.append(qt)
            return q_t, k_t

        def qkv_v(wt, h1):
            v8 = []
            for tt in range(TCH // P):
                ps = pmm.tile([P, C], F32, tag="mm", name="mmps")
                for cc in range(NCC):
                    nc.tensor.matmul(ps, h1[cc][:, tt * P:(tt + 1) * P],
                                     wt["wv"][cc][:, :], start=(cc == 0),
                                     stop=(cc == NCC - 1))
                vt = pv.tile([P, H, HD], F16, tag=f"v{tt}", name=f"vt{tt}")
                if zero_attn_bias:
                    nc.scalar.copy(vt, ps[:].rearrange("p (h d) -> p h d", h=H))
                else:
                    nc.vector.tensor_add(
                        vt, ps[:].rearrange("p (h d) -> p h d", h=H),
                        wt["bv_bc"][:].rearrange("p (h d) -> p h d", h=H))
                v8.append(vt)
            return v8

        def ffn1_emit(wt, h2):
            # alternate PSUM pools: attention's o-banks are idle during FFN,
            # giving FFN1 an effective 4-bank ring
            ffn1 = []
            for fc in range(NFF):
                pool = pmm if fc % 2 == 0 else pops
                ps = pool.tile([P, TCH], F32, tag="mm" if fc % 2 == 0 else "o",
                               name="mmps")
                for cc in range(NCC):
                    nc.tensor.matmul(ps, wt["w1"][cc][:, fc * P:(fc + 1) * P],
                                     h2[cc][:, :], start=(cc == 0),
                                     stop=(cc == NCC - 1))
                ft = pffn.tile([P, TCH], F16, tag=f"f{fc}", name=f"ft{fc}")
                if zero_mlp_bias:
                    nc.scalar.activation(ft, ps, AF.Relu)
                else:
                    nc.scalar.activation(ft, ps, AF.Relu,
                                         bias=wt["b1"][:, fc:fc + 1],
                                         scale=1.0)
                ffn1.append(ft)
            return ffn1

        def ffn2_emit(wt, ffn1, tsl):
            for cc in range(NCC):
                ps = pmm.tile([P, TCH], F32, tag="mm", name="mmps")
                for fc in range(NFF):
                    nc.tensor.matmul(ps, wt["w2"][fc][:, cc * P:(cc + 1) * P],
                                     ffn1[fc][:, :], start=(fc == 0),
                                     stop=(fc == NFF - 1))
                if zero_mlp_bias:
                    nc.vector.tensor_add(x_sb[cc][:, tsl], ps,
                                         x_sb[cc][:, tsl])
                else:
                    nc.vector.scalar_tensor_tensor(
                        x_sb[cc][:, tsl], ps, wt["b2"][:, cc:cc + 1],
                        x_sb[cc][:, tsl], op0=ADD, op1=ADD)

        # ---------------- main loop ----------------
        # LN1 state for the upcoming chunk (a/b in SBUF), produced one
        # chunk ahead; h1 applied during the previous chunk's FFN phase.
        wt = load_weights(0)
        wt_next = None
        S0, S1 = ln_stats(slice(0, TCH))
        ab = ln_finish(S0, S1)
        a1, b1 = ln_bcast(*ab, "ln1", evac=True)
        h1, h8 = ln_apply(slice(0, TCH), a1, b1, "h")

        for l in range(L):
            for ti in range(NTC):
                tsl = slice(ti * TCH, (ti + 1) * TCH)
                mark("qk")
                q_t, k_t = qkv_qk(wt, h8)
                mark("v")
                v8 = qkv_v(wt, h1)
                # prefetch next layer's weights early in the layer
                if ti == 1 and l + 1 < L:
                    wt_next = load_weights(l + 1)
                # 3. attention: scores run 2-3 heads ahead of o/den so PE
                # never waits on the exp round-trip; bi1 scores and bi0 proj
                # fill the den-reciprocal/normalize latencies.
                mark("attn")
                o_t = [po.tile([P, TCH], F16, tag=f"o{hq}", name=f"ot{hq}")
                       for hq in range(NCC)]
                # --- bi0 ---
                den0 = pden.tile([H, T], F32, tag="den", name="den_ps")
                e_q = []
                for hh in range(H):
                    e_q.append(attn_scores(q_t, k_t, 0, hh))
                    if hh >= 3:
                        attn_odens(v8, e_q.pop(0), o_t, den0, 0, hh - 3)
                # interleave bi0 drain with bi1 scores (odens first so the
                # sc ring slot frees before the next scores allocation)
                attn_odens(v8, e_q.pop(0), o_t, den0, 0, H - 3)
                e_q.append(attn_scores(q_t, k_t, 1, 0))
                attn_odens(v8, e_q.pop(0), o_t, den0, 0, H - 2)
                e_q.append(attn_scores(q_t, k_t, 1, 1))
                attn_odens(v8, e_q.pop(0), o_t, den0, 0, H - 1)
                e_q.append(attn_scores(q_t, k_t, 1, 2))
                rden0 = attn_recip(den0)
                den1 = pden.tile([H, T], F32, tag="den", name="den_ps")
                attn_odens(v8, e_q.pop(0), o_t, den1, 1, 0)
                e_q.append(attn_scores(q_t, k_t, 1, 3))
                attn_rdbmul(o_t, rden0, 0)
                for hh in range(4, H):
                    attn_odens(v8, e_q.pop(0), o_t, den1, 1, hh - 3)
                    e_q.append(attn_scores(q_t, k_t, 1, hh))
                # warmup: pay the sqrt act-table swap now (off the LN2
                # critical path; remaining attention Act ops are copies,
                # which live in every table set)
                nc.scalar.activation(warm_t, eps_t, AF.Sqrt)
                for phh in (H - 3, H - 2, H - 1):
                    attn_odens(v8, e_q.pop(0), o_t, den1, 1, phh)
                rden1 = attn_recip(den1)
                # 4. proj bi0 + its half of LN2 stats fill PE while bi1
                # normalizes; LN1' stats for the next chunk fill the LN2
                # finish latency; bcast1'/apply1' ride behind FFN1/FFN2.
                mark("proj")
                proj_bi(wt, o_t, tsl, 0)
                S0 = psc.tile([1, TCH], F32, tag="sc", name="S0")
                S1 = psc.tile([1, TCH], F32, tag="sc", name="S1")
                fin2 = ln_alloc_fin()
                ln_stats_bi(tsl, S0, S1, 0)
                ln_finish_half(S0, S1, *fin2, slice(0, T))
                attn_rdbmul(o_t, rden1, 1)
                proj_bi(wt, o_t, tsl, 1)
                mark("ln")
                ln_stats_bi(tsl, S0, S1, 1)
                ln_finish_half(S0, S1, *fin2, slice(T, TCH))
                ab2 = (fin2[2], fin2[3])
                last = (l == L - 1 and ti == NTC - 1)
                a2, b2 = ln_bcast(*ab2, "ln2")
                if not last:
                    ntsl = slice(((ti + 1) % NTC) * TCH,
                                 (((ti + 1) % NTC) + 1) * TCH)
                    nS0, nS1 = ln_stats(ntsl, pool=pmm)
                h2 = ln_apply(tsl, a2, b2, "g")
                if not last:
                    nab = ln_finish(nS0, nS1)
                mark("ffn1")
                ffn1 = ffn1_emit(wt, h2)
                if not last:
                    a1, b1 = ln_bcast(*nab, "ln1", evac=True)
                    h1, h8 = ln_apply(ntsl, a1, b1, "h")
                # 12. FFN2 + residual
                mark("ffn2")
                ffn2_emit(wt, ffn1, tsl)
            if l + 1 < L:
                wt = wt_next

        mark("lmhead")
        # ---------------- lm head (fp32r) ----------------
        for tt in range(NT // P):
            ps = pmm.tile([P, V], F32, tag="mm", name="mmps")
            for cc in range(NCC):
                nc.tensor.matmul(ps, x_sb[cc][:, tt * P:(tt + 1) * P],
                                 wlm_sb[cc][:, :], start=(cc == 0),
                                 stop=(cc == NCC - 1))
            lo = plog.tile([P, V], F32, tag="lg", name="lo")
            nc.vector.tensor_add(lo, ps, blm_bc)
            nc.sync.dma_start(out=out_d[tt * P:(tt + 1) * P, :], in_=lo)

    if not nc.is_finalized():
        nc.finalize()
    return nc


def prep_inputs(idx, tok_emb, pos_emb, Wq, Wk, Wv, Wo, bo, ln1_g, ln1_b,
                ln2_g, ln2_b, W1, b1, W2, b2, Wlm, blm):
    """host-side: fold LN affines into weights, build per-core input maps"""
    f32 = np.float32
    idx = np.asarray(idx)
    tok_emb = np.asarray(tok_emb, f32)
    pos_emb = np.asarray(pos_emb, f32)
    scale = C ** -0.5

    wq = np.empty((L, C, C), f32)
    wk = np.empty((L, C, C), f32)
    wv = np.empty((L, C, C), f32)
    wo = np.empty((L, C, C), f32)
    w1 = np.empty((L, C, FF), f32)
    w2 = np.empty((L, FF, C), f32)
    bq = np.empty((L, C), f32)
    bk = np.empty((L, C), f32)
    bv = np.empty((L, C), f32)
    b1f = np.empty((L, FF), f32)
    for l in range(L):
        wq_c = np.asarray(Wq[l], f32).transpose(1, 0, 2).reshape(C, C)
        wk_c = np.asarray(Wk[l], f32).transpose(1, 0, 2).reshape(C, C)
        wv_c = np.asarray(Wv[l], f32).transpose(1, 0, 2).reshape(C, C)
        g1 = np.asarray(ln1_g[l], f32)[:, None]
        b1_ = np.asarray(ln1_b[l], f32)
        g2 = np.asarray(ln2_g[l], f32)[:, None]
        b2_ = np.asarray(ln2_b[l], f32)
        wq[l] = g1 * wq_c * scale
        bq[l] = (b1_ @ wq_c) * scale
        wk[l] = g1 * wk_c
        bk[l] = b1_ @ wk_c
        wv[l] = g1 * wv_c
        bv[l] = b1_ @ wv_c
        wo[l] = np.asarray(Wo[l], f32)
        w1[l] = g2 * np.asarray(W1[l], f32)
        b1f[l] = np.asarray(b1[l], f32) + b2_ @ np.asarray(W1[l], f32)
        w2[l] = np.asarray(W2[l], f32)

    bo = np.asarray(bo, f32)
    b2a = np.asarray(b2, f32)

    # additive causal mask, [s%128, s//128, t] layout, -30 where masked
    s_g = np.arange(2 * P).reshape(2, P).T          # [128, 2] global s
    mall = ((s_g[:, :, None] > np.arange(T)[None, None, :])
            .astype(np.float16) * np.float16(-30.0))
    i16 = np.eye(P, dtype=np.float16)

    pos2 = np.concatenate([pos_emb.T, pos_emb.T], axis=1)  # [C, 512]

    flags = {
        "zero_attn_bias": not (np.any(bq) or np.any(bk) or np.any(bv)
                               or np.any(bo)),
        "zero_mlp_bias": not (np.any(b1f) or np.any(b2a)),
    }

    e8sel = np.zeros((NCC, H, P), f32)
    for hq in range(NCC):
        for p_ in range(P):
            e8sel[hq, 2 * hq + p_ // HD, p_] = 1.0

    import ml_dtypes
    f8 = ml_dtypes.float8_e4m3fn

    def pack8(w, s):
        assert np.abs(w).max() * s < 440.0, "fp8 prescale overflow"
        pk = (w * s).reshape(L, 2, 2, P, C).transpose(0, 1, 3, 2, 4)
        return np.ascontiguousarray(pk).astype(f8)

    shared = {
        "e8sel": e8sel,
        "tok_emb16": tok_emb.astype(np.float16),
        "pos2T": np.ascontiguousarray(pos2).astype(np.float16),
        "i16": i16, "mall": np.ascontiguousarray(mall),
        "wq8": pack8(wq, QS8), "wk8": pack8(wk, KS8),
        "wv": wv.astype(np.float16), "wo": wo.astype(np.float16),
        "w1": w1.astype(np.float16), "w2": w2.astype(np.float16),
        "bq": bq, "bk": bk, "bv": bv,
        "bo": bo, "b1": b1f, "b2": b2a,
        "wlm": np.asarray(Wlm, f32), "blm": np.asarray(blm, f32),
    }
    in_maps = []
    vocab = np.arange(V)
    for core in range(NCORES):
        toks = np.asarray(idx[core * BL:(core + 1) * BL]).reshape(-1)
        oh = (vocab[:, None] == toks[None, :]).astype(np.float16)
        m = dict(shared)
        m["onehotT"] = np.ascontiguousarray(oh)
        in_maps.append(m)
    return in_maps, flags


_NC_CACHE = {}


def get_nc(flags=None):
    if flags is None:
        flags = {"zero_attn_bias": False, "zero_mlp_bias": False}
    key = (flags["zero_attn_bias"], flags["zero_mlp_bias"])
    if key not in _NC_CACHE:
        _NC_CACHE[key] = build_bass(**flags)
    return _NC_CACHE[key]


def run(in_maps, flags=None, trace=False, **kw):
    from concourse.bass_utils import run_bass_kernel_spmd
    nc = get_nc(flags)
    return run_bass_kernel_spmd(nc, in_maps, list(range(NCORES)),
                                trace=trace, **kw)


def kernel(**inputs):
    in_maps, flags = prep_inputs(**inputs)
    res = run(in_maps, flags)
    outs = [res.results[i]["out"].reshape(BL, T, V) for i in range(NCORES)]
    return np.concatenate(outs, axis=0).astype(np.float32)


# revision 32
# speedup vs baseline: 1.6537x; 1.6537x over previous
"""Bass/Tile TRN2 kernel for nn_BigramLanguageModel (8-layer dense transformer).

Data-parallel over batch across 8 NeuronCores (8 items/core, no collectives).
Residual stream kept feature-major ([C, tokens]) in SBUF so every matmul
contracts over the partition dim without transposes.

v2 restructure (vs v1): the per-head attention chain is PE->Act->PE only —
the causal mask is folded into the scores PSUM group as small constant
matmuls (-30 triangles; exp(-30) flushes to 0 in fp16), so each head needs
ONE exp activation and no GPSIMD/DVE mask op. Heads are emitted with a
sliding window (scores run 2 heads ahead of o/den) so PE never waits on the
Act exp round-trip. LN chains are software-pipelined: LN1 of chunk ti+1 is
computed (stats/finish/bcast during the LN2 DVE latency of chunk ti, apply
during FFN) so PE's dense stream is uninterrupted. Weights double-buffered
across layers.
"""

import os
import sys
from contextlib import ExitStack

import numpy as np

for _p in ("/opt/trn_rl_repo", "/root/.axon_site/_ro/trn_rl_repo"):
    if os.path.isdir(_p) and _p not in sys.path:
        sys.path.insert(0, _p)
        break

import concourse.bass as bass
import concourse.mybir as mybir
import concourse.tile as tile
from concourse import bacc

# model config (hardcoded per problem spec)
B, T, C, H, L, V = 64, 256, 512, 8, 8, 100
HD = C // H          # 64
FF = 4 * C           # 2048
EPS = 1e-5
NCORES = 8
BL = B // NCORES     # 8 batch items per core
NT = BL * T          # 2048 tokens per core
P = 128
NCC = C // P         # 4 c-chunks
NFF = FF // P        # 16 ff-chunks
TCH = 512            # token chunk (2 batch items)
NTC = NT // TCH      # 4
BI = TCH // T        # 2 batch items per token chunk

QS8 = 2048.0   # fp8 pre-scale for folded Wq (tiny: has 1/sqrt(C) baked in)
KS8 = 64.0     # fp8 pre-scale for folded Wk
F32 = mybir.dt.float32
F16 = mybir.dt.float16
F32R = mybir.dt.float32r
ADD = mybir.AluOpType.add
MULT = mybir.AluOpType.mult
AF = mybir.ActivationFunctionType


def _r(ap):
    return ap if ap.dtype == F32R else ap.bitcast(F32R)


def _bcast_dram(vec_ap, parts):
    return bass.AP(
        tensor=vec_ap.tensor,
        offset=vec_ap.offset,
        ap=[[0, parts]] + [list(d) for d in vec_ap.ap],
    )


PHASE_MARKS = []


def build_bass(zero_attn_bias=False, zero_mlp_bias=False):
    nc = bacc.Bacc()
    PHASE_MARKS.clear()

    def mark(label):
        PHASE_MARKS.append(
            (label, int(nc.get_next_instruction_name().split("-")[1])))
    dp = nc.declare_dram_parameter

    onehot_d = dp("onehotT", [V, NT], F16, False)
    tok_d = dp("tok_emb16", [V, C], F16, False)
    pos16_d = dp("pos2T", [C, TCH], F16, False)
    tri01_d = dp("tri01", [P, P], F16, False)    # 1 where s<=t (in-block)
    wq8_d = dp("wq8", [L, 2, P, 2, C], mybir.dt.float8e4, False)
    wk8_d = dp("wk8", [L, 2, P, 2, C], mybir.dt.float8e4, False)
    wv_d = dp("wv", [L, C, C], F16, False)
    wo_d = dp("wo", [L, C, C], F16, False)
    w1_d = dp("w1", [L, C, FF], F16, False)
    w2_d = dp("w2", [L, FF, C], F16, False)
    bq_d = dp("bq", [L, C], F32, False)
    bk_d = dp("bk", [L, C], F32, False)
    bv_d = dp("bv", [L, C], F32, False)
    bo_d = dp("bo", [L, C], F32, False)
    b1_d = dp("b1", [L, FF], F32, False)
    b2_d = dp("b2", [L, C], F32, False)
    e8_d = dp("e8sel", [NCC, H, P], F32, False)
    wlm_d = dp("wlm", [C, V], F32, False)
    blm_d = dp("blm", [V], F32, False)
    out_d = dp("out", [NT, V], F32, True)

    with tile.TileContext(nc) as tc, ExitStack() as ctx:
        # ---------------- pools ----------------
        pconst = ctx.enter_context(tc.tile_pool(name="const", bufs=1))
        px = ctx.enter_context(tc.tile_pool(name="x", bufs=1))
        pw = ctx.enter_context(tc.tile_pool(name="w", bufs=1))
        pwq = ctx.enter_context(tc.tile_pool(name="wq", bufs=2))
        pbias = ctx.enter_context(tc.tile_pool(name="bias", bufs=2))
        ph = ctx.enter_context(tc.tile_pool(name="h", bufs=1))
        pq = ctx.enter_context(tc.tile_pool(name="q", bufs=1))
        pv = ctx.enter_context(tc.tile_pool(name="v", bufs=1))
        po = ctx.enter_context(tc.tile_pool(name="o", bufs=1))
        pffn = ctx.enter_context(tc.tile_pool(name="ffn", bufs=1))
        psq = ctx.enter_context(tc.tile_pool(name="sq", bufs=2))
        pstat = ctx.enter_context(tc.tile_pool(name="stat", bufs=2))
        pln = ctx.enter_context(tc.tile_pool(name="ln", bufs=1))
        pe_ = ctx.enter_context(tc.tile_pool(name="e", bufs=6))
        prd = ctx.enter_context(tc.tile_pool(name="rd", bufs=2))
        plog = ctx.enter_context(tc.tile_pool(name="log", bufs=2))
        # PSUM: 8 banks total
        pmm = ctx.enter_context(tc.tile_pool(name="mm", bufs=2, space="PSUM"))
        psc = ctx.enter_context(tc.tile_pool(name="scps", bufs=3, space="PSUM"))
        pops = ctx.enter_context(tc.tile_pool(name="ops", bufs=2, space="PSUM"))
        pden = ctx.enter_context(tc.tile_pool(name="den", bufs=1, space="PSUM"))

        # ---------------- constants ----------------
        # ones vector pre-scaled by 1/C so LN stats matmuls emit means
        ones_f = pconst.tile([P, 1], F32, tag="ones_f", name="ones_f")
        nc.vector.memset(ones_f, 1.0 / C)
        ones = pconst.tile([P, 1], F32R, tag="ones", name="ones")
        nc.vector.tensor_copy(ones, ones_f)
        ones16 = pconst.tile([P, 1], F16, tag="ones16", name="ones16")
        nc.vector.tensor_copy(ones16, ones_f)
        ones1_f = pconst.tile([1, P], F32, tag="ones1_f", name="ones1_f")
        nc.vector.memset(ones1_f, 1.0)
        ones1 = pconst.tile([1, P], F32R, tag="ones1", name="ones1")
        nc.vector.tensor_copy(ones1, ones1_f)
        eps_t = pconst.tile([1, 1], F32, tag="eps", name="eps")
        nc.vector.memset(eps_t, EPS)
        warm_t = pconst.tile([1, 1], F32, tag="warm", name="warm")
        tri01 = pconst.tile([P, P], F16, tag="tri01", name="tri01")
        nc.sync.dma_start(out=tri01, in_=tri01_d[:, :])
        tok_sb = pconst.tile([V, C], F16, tag="tok", name="tok")
        nc.sync.dma_start(out=tok_sb, in_=tok_d[:, :])
        wlm_sb = []
        for cc in range(NCC):
            f = pconst.tile([P, V], F32, tag=f"wlmf{cc}", name=f"wlmf{cc}")
            nc.sync.dma_start(out=f, in_=wlm_d[cc * P:(cc + 1) * P, :])
            t = pconst.tile([P, V], F32R, tag=f"wlm{cc}", name=f"wlm{cc}")
            nc.vector.tensor_copy(t, f)
            wlm_sb.append(t)
        blm_bc = pconst.tile([P, V], F32, tag="blm", name="blm")
        nc.sync.dma_start(out=blm_bc, in_=_bcast_dram(blm_d[:], P))
        e8 = []
        for hq in range(NCC):
            f = pconst.tile([H, P], F32, tag=f"e8f{hq}", name=f"e8f{hq}")
            nc.sync.dma_start(out=f, in_=e8_d[hq])
            r8 = pconst.tile([H, P], F32R, tag=f"e8{hq}", name=f"e8{hq}")
            nc.vector.tensor_copy(r8, f)
            e8.append(r8)
        # one-hot columns for denominator matmuls: o8c[p, hh, j] = (j == hh)
        o8c = pconst.tile([P, H, H], F16, tag="o8c", name="o8c")
        nc.vector.memset(o8c, 0.0)
        for hh in range(H):
            nc.vector.memset(o8c[:, hh, hh:hh + 1], 1.0)

        # resident residual stream, feature-major: x_T[c, t]
        x_sb = [px.tile([P, NT], F32R, tag=f"x{cc}", name=f"x{cc}")
                for cc in range(NCC)]

        # ---------------- embedding ----------------
        with tc.tile_pool(name="emb", bufs=1) as pemb:
            oh_sb = pemb.tile([V, NT], F16, tag="oh", name="oh")
            nc.sync.dma_start(out=oh_sb, in_=onehot_d[:, :])
            pos_sb = []
            for cc in range(NCC):
                t = pemb.tile([P, TCH], F16, tag=f"pos{cc}", name=f"pos{cc}")
                nc.sync.dma_start(out=t, in_=pos16_d[cc * P:(cc + 1) * P, :])
                pos_sb.append(t)
            for ti in range(NTC):
                tsl = slice(ti * TCH, (ti + 1) * TCH)
                for cc in range(NCC):
                    ps = pmm.tile([P, TCH], F32, tag="mm", name="mmps")
                    nc.tensor.matmul(ps, tok_sb[:, cc * P:(cc + 1) * P],
                                     oh_sb[:, tsl], start=True, stop=True)
                    nc.vector.tensor_add(x_sb[cc][:, tsl], ps, pos_sb[cc])

        # ---------------- LN building blocks ----------------
        def ln_stats(tsl, pool=None):
            """S0 = mean(x), S1 = mean(x^2) via 1/C-ones matmuls on PE.
            squares computed on the Act engine (fp16) to keep DVE clear."""
            pool = pool or psc
            S0 = pool.tile([1, TCH], F32, tag=pool is psc and "sc" or "mm",
                           name="S0")
            S1 = pool.tile([1, TCH], F32, tag=pool is psc and "sc" or "mm",
                           name="S1")
            for cc in range(NCC):
                sq = psq.tile([P, TCH], F16, tag="sq", name="sq")
                nc.scalar.square(sq, x_sb[cc][:, tsl].bitcast(F32))
                nc.tensor.matmul(S0[0:1, :], _r(ones[:, :]), x_sb[cc][:, tsl],
                                 start=(cc == 0), stop=(cc == NCC - 1))
                nc.tensor.matmul(S1[0:1, :], ones16[:, :], sq[:, :],
                                 start=(cc == 0), stop=(cc == NCC - 1))
            return S0, S1

        def ln_stats_bi(tsl, S0, S1, bi):
            """half-chunk stats: each bi's groups open their own region
            (replace-first on every byte under either PSUM-start model)."""
            bsl = slice(bi * T, (bi + 1) * T)
            xsl = slice(tsl.start + bi * T, tsl.start + (bi + 1) * T)
            for cc in range(NCC):
                sq = psq.tile([P, T], F16, tag="sq", name="sq")
                nc.scalar.square(sq, x_sb[cc][:, xsl].bitcast(F32))
                nc.tensor.matmul(S0[0:1, bsl], _r(ones[:, :]),
                                 x_sb[cc][:, xsl], start=(cc == 0),
                                 stop=(cc == NCC - 1),
                                 skip_group_check=True)
                nc.tensor.matmul(S1[0:1, bsl], ones16[:, :], sq[:, :],
                                 start=(cc == 0), stop=(cc == NCC - 1),
                                 skip_group_check=True)

        def ln_finish_half(S0, S1, m_t, m2_t, v_t, b_t, bsl):
            """finish on one half of the stats row (overlaps the other bi)."""
            nc.vector.tensor_copy(m_t[:, bsl], S0[0:1, bsl])
            nc.vector.tensor_mul(m2_t[:, bsl], m_t[:, bsl], m_t[:, bsl])
            nc.vector.tensor_sub(v_t[:, bsl], S1[0:1, bsl], m2_t[:, bsl])
            nc.scalar.activation(v_t[:, bsl], v_t[:, bsl], AF.Sqrt,
                                 bias=eps_t[:, :], scale=1.0)
            with nc.allow_low_precision("fp32r rstd is fp32-equivalent"):
                nc.vector.reciprocal(v_t[:, bsl], v_t[:, bsl])
            nc.vector.scalar_tensor_tensor(b_t[:, bsl], m_t[:, bsl], -1.0,
                                           v_t[:, bsl], op0=MULT, op1=MULT)

        def ln_alloc_fin():
            m_t = pstat.tile([1, TCH], F32R, tag="m0", name="m_t")
            m2_t = pstat.tile([1, TCH], F32, tag="m2", name="m2_t")
            v_t = pstat.tile([1, TCH], F32R, tag="v", name="v_t")
            b_t = pstat.tile([1, TCH], F32R, tag="m", name="b_t")
            return m_t, m2_t, v_t, b_t

        def ln_finish(S0, S1):
            """A = rstd, B = -mean*rstd (S0 = mean, S1 = E[x^2] already).
            sqrt table set load is prepaid by a warmup op late in the
            attention phase."""
            m_t = pstat.tile([1, TCH], F32R, tag="m0", name="m_t")
            m2_t = pstat.tile([1, TCH], F32, tag="m2", name="m2_t")
            v_t = pstat.tile([1, TCH], F32R, tag="v", name="v_t")
            b_t = pstat.tile([1, TCH], F32R, tag="m", name="b_t")
            nc.vector.tensor_copy(m_t, S0[0:1, :])
            nc.vector.tensor_mul(m2_t, m_t, m_t)
            nc.vector.tensor_sub(v_t, S1[0:1, :], m2_t)
            nc.scalar.activation(v_t, v_t, AF.Sqrt, bias=eps_t[:, :], scale=1.0)
            with nc.allow_low_precision("fp32r rstd is fp32-equivalent"):
                nc.vector.reciprocal(v_t, v_t)
            nc.vector.scalar_tensor_tensor(b_t, m_t, -1.0, v_t,
                                           op0=MULT, op1=MULT)
            return v_t, b_t

        def ln_bcast(v_t, m_t, atag, evac=False):
            """broadcast A,B rows to [P, TCH] PSUM; LN2 applies straight from
            PSUM (critical path), LN1-next evacuates to SBUF off-path so the
            psc ring is free during FFN"""
            a_ps = psc.tile([P, TCH], F32, tag="sc", name="a_ps")
            nc.tensor.matmul(a_ps, _r(ones1[:, :]), v_t[:, :],
                             start=True, stop=True)
            b_ps = psc.tile([P, TCH], F32, tag="sc", name="b_ps")
            nc.tensor.matmul(b_ps, _r(ones1[:, :]), m_t[:, :],
                             start=True, stop=True)
            if not evac:
                return a_ps, b_ps
            a_sb = pln.tile([P, TCH], F32, tag=f"{atag}a", name=f"{atag}a")
            b_sb = pln.tile([P, TCH], F32, tag=f"{atag}b", name=f"{atag}b")
            nc.vector.tensor_copy(a_sb, a_ps)
            nc.vector.tensor_copy(b_sb, b_ps)
            return a_sb, b_sb

        def ln_apply(tsl, a_sb, b_sb, htag):
            h = []
            h8 = [ph.tile([P, 2, TCH], mybir.dt.float8e4, tag=f"{htag}8{kp}",
                          name=f"h8{kp}") for kp in range(2)] \
                if htag == "h" else None
            for cc in range(NCC):
                d = ph.tile([P, TCH], F16, tag=f"{htag}{cc}", name=f"h{cc}")
                nc.vector.tensor_mul(d, x_sb[cc][:, tsl], a_sb)
                nc.vector.tensor_add(d, d, b_sb)
                if h8 is not None:
                    nc.scalar.copy(h8[cc // 2][:, cc % 2, :], d)
                h.append(d)
            if h8 is not None:
                return h, h8
            return h

        # ---------------- per-layer weights ----------------
        def load_weights(l):
            def _load(dram, tag, n, width, pool=pw):
                ts_ = []
                for i in range(n):
                    t = pool.tile([P, width], F16, tag=f"{tag}{i}", name=f"{tag}{i}")
                    nc.sync.dma_start(out=t, in_=dram[l, i * P:(i + 1) * P, :])
                    ts_.append(t)
                return ts_

            w = {}
            # qkv double-buffered (needed right at layer start); the rest
            # single-buffered (ample DMA slack before first use)
            F8 = mybir.dt.float8e4
            for nm, dr in (("wq8", wq8_d), ("wk8", wk8_d)):
                ts_ = []
                for kp in range(2):
                    t = pwq.tile([P, 2, C], F8, tag=f"{nm}{kp}",
                                 name=f"{nm}{kp}")
                    nc.sync.dma_start(out=t, in_=dr[l, kp])
                    ts_.append(t)
                w[nm] = ts_
            w["wv"] = _load(wv_d, "wv", NCC, C, pwq)
            w["wo"] = _load(wo_d, "wo", NCC, C)
            w["w1"] = _load(w1_d, "w1", NCC, FF)
            w["w2"] = _load(w2_d, "w2", NFF, C)
            if not zero_attn_bias:
                for nm, dr in (("bq", bq_d), ("bk", bk_d), ("bo", bo_d)):
                    t = pbias.tile([P, NCC], F32, tag=nm, name=nm)
                    nc.sync.dma_start(out=t, in_=dr[l].rearrange("(a p) -> p a", p=P))
                    w[nm] = t
                bv_bc = pbias.tile([P, C], F32, tag="bvb", name="bvb")
                nc.sync.dma_start(out=bv_bc, in_=_bcast_dram(bv_d[l], P))
                w["bv_bc"] = bv_bc
            if not zero_mlp_bias:
                t = pbias.tile([P, NFF], F32, tag="b1", name="b1")
                nc.sync.dma_start(out=t, in_=b1_d[l].rearrange("(a p) -> p a", p=P))
                w["b1"] = t
                t = pbias.tile([P, NCC], F32, tag="b2", name="b2")
                nc.sync.dma_start(out=t, in_=b2_d[l].rearrange("(a p) -> p a", p=P))
                w["b2"] = t
            return w

        # ---------------- attention per (bi, head) ----------------
        def attn_scores(q_t, k_t, bi, hh):
            """scores + mask into one PSUM bank; returns (sc_ps,)"""
            hq, hr = divmod(hh, 2)
            rsl = slice(hr * HD, (hr + 1) * HD)
            qsl = q_t[hq][rsl, bi * T:(bi + 1) * T]
            ksl0 = k_t[hq][rsl, bi * T: bi * T + P]
            ksl1 = k_t[hq][rsl, bi * T + P: bi * T + 2 * P]
            sc = psc.tile([P, 2, T], F32, tag="sc", name="sc")
            # scores only (no mask matmuls); each region opened by its own
            # start=True; the fully-masked [.,1,0:128] region is never
            # computed or read. diagonal triangles are zeroed multiplicatively
            # on the otherwise-idle GPSIMD engine after the exp.
            nc.tensor.matmul(sc[:, 0, :], ksl0, qsl, start=True, stop=False)
            nc.tensor.matmul(sc[:, 1, P:T], ksl1, qsl[:, P:T],
                             start=True, stop=True, skip_group_check=True)
            e = pe_.tile([P, 2, T], F16, tag="e", name="e")
            nc.scalar.activation(e[:, 0, :], sc[:, 0, :], AF.Exp)
            nc.scalar.activation(e[:, 1, P:T], sc[:, 1, P:T], AF.Exp)
            nc.gpsimd.tensor_mul(e[:, 0, 0:P], e[:, 0, 0:P], tri01[:, :])
            nc.gpsimd.tensor_mul(e[:, 1, P:T], e[:, 1, P:T], tri01[:, :])
            return e

        def attn_odens(v8, e, o_t, den_ps, bi, hh):
            hq, hr = divmod(hh, 2)
            rsl = slice(hr * HD, (hr + 1) * HD)
            o_ps = pops.tile([HD, T], F32, tag="o", name="o_ps")
            nc.tensor.matmul(o_ps, v8[bi * 2][:, hh, :], e[:, 0, :],
                             start=True, stop=False)
            nc.tensor.matmul(o_ps[:, P:T], v8[bi * 2 + 1][:, hh, :],
                             e[:, 1, P:T], start=False, stop=True)
            nc.tensor.matmul(den_ps, o8c[:, hh, :], e[:, 0, :],
                             start=(hh == 0), stop=False)
            nc.tensor.matmul(den_ps[:, P:T], o8c[:, hh, :], e[:, 1, P:T],
                             start=False, stop=(hh == H - 1))
            nc.scalar.copy(o_t[hq][rsl, bi * T:(bi + 1) * T], o_ps[0:HD, :])

        def attn_recip(den_ps):
            rden = prd.tile([H, T], F32R, tag="rden", name="rden")
            with nc.allow_low_precision("fp32r rden is fp32-equivalent"):
                nc.vector.reciprocal(rden, den_ps)
            return rden

        def attn_rdbmul(o_t, rden, bi):
            for hq in range(NCC):
                rdb = pmm.tile([P, T], F32, tag="mm", name="rdb")
                nc.tensor.matmul(rdb, e8[hq][:, :], rden[:, :],
                                 start=True, stop=True)
                osl = o_t[hq][:, bi * T:(bi + 1) * T]
                nc.vector.tensor_mul(osl, osl, rdb)

        def proj_bi(wt, o_t, tsl, bi):
            """proj for one batch item; accumulate into x + LN2 stats gate"""
            bsl = slice(bi * T, (bi + 1) * T)
            xsl = slice(tsl.start + bi * T, tsl.start + (bi + 1) * T)
            for cc in range(NCC):
                pool = pmm if cc % 2 == 0 else psc
                ps = pool.tile([P, T], F32,
                               tag="mm" if pool is pmm else "sc",
                               name="prps")
                for hq in range(NCC):
                    nc.tensor.matmul(ps, wt["wo"][hq][:, cc * P:(cc + 1) * P],
                                     o_t[hq][:, bsl], start=(hq == 0),
                                     stop=(hq == NCC - 1))
                if zero_attn_bias:
                    nc.vector.tensor_add(x_sb[cc][:, xsl], ps,
                                         x_sb[cc][:, xsl])
                else:
                    nc.vector.scalar_tensor_tensor(
                        x_sb[cc][:, xsl], ps, wt["bo"][:, cc:cc + 1],
                        x_sb[cc][:, xsl], op0=ADD, op1=ADD)

        # ---------------- qkv ----------------
        def qkv_qk(wt, h8):
            """q,k via fp8e4 DoubleRow (K=256 per instruction); the weight
            pre-scales (QS8/KS8) are undone in the PSUM evacuation"""
            DRM = mybir.MatmulPerfMode.DoubleRow
            q_t, k_t = [], []
            gi = 0
            for dst, wnm, bnm, usc in ((q_t, "wq8", "bq", 1.0 / QS8),
                                       (k_t, "wk8", "bk", 1.0 / KS8)):
                for hq in range(NCC):
                    # alternate PSUM pools: short DR groups would otherwise
                    # serialize on the 2-bank ring's evacuation latency
                    pool = pmm if gi % 2 == 0 else pops
                    gi += 1
                    ps = pool.tile([P, TCH], F32,
                                   tag="mm" if pool is pmm else "o",
                                   name="mmps")
                    for kp in range(2):
                        nc.tensor.matmul(
                            ps, wt[wnm][kp][:, :, hq * P:(hq + 1) * P],
                            h8[kp][:, :, :], start=(kp == 0),
                            stop=(kp == 1), perf_mode=DRM)
                    qt = pq.tile([P, TCH], F16, tag=f"{bnm}t{hq}",
                                 name=f"{bnm}t{hq}")
                    if zero_attn_bias:
                        nc.scalar.mul(qt, ps, usc)
                    else:
                        nc.scalar.activation(qt, ps, AF.Identity,
                                             bias=wt[bnm][:, hq:hq + 1],
                                             scale=usc)
                    dst.append(qt)
            return q_t, k_t

        def qkv_v(wt, h1):
            v8 = []
            for tt in range(TCH // P):
                ps = pmm.tile([P, C], F32, tag="mm", name="mmps")
                for cc in range(NCC):
                    nc.tensor.matmul(ps, h1[cc][:, tt * P:(tt + 1) * P],
                                     wt["wv"][cc][:, :], start=(cc == 0),
                                     stop=(cc == NCC - 1))
                vt = pv.tile([P, H, HD], F16, tag=f"v{tt}", name=f"vt{tt}")
                if zero_attn_bias:
                    nc.scalar.copy(vt, ps[:].rearrange("p (h d) -> p h d", h=H))
                else:
                    nc.vector.tensor_add(
                        vt, ps[:].rearrange("p (h d) -> p h d", h=H),
                        wt["bv_bc"][:].rearrange("p (h d) -> p h d", h=H))
                v8.append(vt)
            return v8

        def ffn1_emit(wt, h2):
            # alternate PSUM pools: attention's o-banks are idle during FFN,
            # giving FFN1 an effective 4-bank ring
            ffn1 = []
            for fc in range(NFF):
                pool = pmm if fc % 2 == 0 else pops
                ps = pool.tile([P, TCH], F32, tag="mm" if fc % 2 == 0 else "o",
                               name="mmps")
                for cc in range(NCC):
                    nc.tensor.matmul(ps, wt["w1"][cc][:, fc * P:(fc + 1) * P],
                                     h2[cc][:, :], start=(cc == 0),
                                     stop=(cc == NCC - 1))
                ft = pffn.tile([P, TCH], F16, tag=f"f{fc}", name=f"ft{fc}")
                if zero_mlp_bias:
                    nc.scalar.activation(ft, ps, AF.Relu)
                else:
                    nc.scalar.activation(ft, ps, AF.Relu,
                                         bias=wt["b1"][:, fc:fc + 1],
                                         scale=1.0)
                ffn1.append(ft)
            return ffn1

        def ffn2_emit(wt, ffn1, tsl):
            for cc in range(NCC):
                ps = pmm.tile([P, TCH], F32, tag="mm", name="mmps")
                for fc in range(NFF):
                    nc.tensor.matmul(ps, wt["w2"][fc][:, cc * P:(cc + 1) * P],
                                     ffn1[fc][:, :], start=(fc == 0),
                                     stop=(fc == NFF - 1))
                if zero_mlp_bias:
                    nc.vector.tensor_add(x_sb[cc][:, tsl], ps,
                                         x_sb[cc][:, tsl])
                else:
                    nc.vector.scalar_tensor_tensor(
                        x_sb[cc][:, tsl], ps, wt["b2"][:, cc:cc + 1],
                        x_sb[cc][:, tsl], op0=ADD, op1=ADD)

        # ---------------- main loop ----------------
        # LN1 state for the upcoming chunk (a/b in SBUF), produced one
        # chunk ahead; h1 applied during the previous chunk's FFN phase.
        wt = load_weights(0)
        wt_next = None
        S0, S1 = ln_stats(slice(0, TCH))
        ab = ln_finish(S0, S1)
        a1, b1 = ln_bcast(*ab, "ln1", evac=True)
        h1, h8 = ln_apply(slice(0, TCH), a1, b1, "h")

        for l in range(L):
            for ti in range(NTC):
                tsl = slice(ti * TCH, (ti + 1) * TCH)
                mark("qk")
                q_t, k_t = qkv_qk(wt, h8)
                mark("v")
                v8 = qkv_v(wt, h1)
                # prefetch next layer's weights early in the layer
                if ti == 1 and l + 1 < L:
                    wt_next = load_weights(l + 1)
                # 3. attention: scores run 2-3 heads ahead of o/den so PE
                # never waits on the exp round-trip; bi1 scores and bi0 proj
                # fill the den-reciprocal/normalize latencies.
                mark("attn")
                o_t = [po.tile([P, TCH], F16, tag=f"o{hq}", name=f"ot{hq}")
                       for hq in range(NCC)]
                # --- bi0 ---
                den0 = pden.tile([H, T], F32, tag="den", name="den_ps")
                e_q = []
                for hh in range(H):
                    e_q.append(attn_scores(q_t, k_t, 0, hh))
                    if hh >= 3:
                        attn_odens(v8, e_q.pop(0), o_t, den0, 0, hh - 3)
                # interleave bi0 drain with bi1 scores (odens first so the
                # sc ring slot frees before the next scores allocation)
                attn_odens(v8, e_q.pop(0), o_t, den0, 0, H - 3)
                e_q.append(attn_scores(q_t, k_t, 1, 0))
                attn_odens(v8, e_q.pop(0), o_t, den0, 0, H - 2)
                e_q.append(attn_scores(q_t, k_t, 1, 1))
                attn_odens(v8, e_q.pop(0), o_t, den0, 0, H - 1)
                e_q.append(attn_scores(q_t, k_t, 1, 2))
                rden0 = attn_recip(den0)
                den1 = pden.tile([H, T], F32, tag="den", name="den_ps")
                attn_odens(v8, e_q.pop(0), o_t, den1, 1, 0)
                e_q.append(attn_scores(q_t, k_t, 1, 3))
                attn_rdbmul(o_t, rden0, 0)
                for hh in range(4, H):
                    attn_odens(v8, e_q.pop(0), o_t, den1, 1, hh - 3)
                    e_q.append(attn_scores(q_t, k_t, 1, hh))
                # warmup: pay the sqrt act-table swap now (off the LN2
                # critical path; remaining attention Act ops are copies,
                # which live in every table set)
                nc.scalar.activation(warm_t, eps_t, AF.Sqrt)
                for phh in (H - 3, H - 2, H - 1):
                    attn_odens(v8, e_q.pop(0), o_t, den1, 1, phh)
                rden1 = attn_recip(den1)
                # 4. proj bi0 + its half of LN2 stats fill PE while bi1
                # normalizes; LN1' stats for the next chunk fill the LN2
                # finish latency; bcast1'/apply1' ride behind FFN1/FFN2.
                mark("proj")
                proj_bi(wt, o_t, tsl, 0)
                S0 = psc.tile([1, TCH], F32, tag="sc", name="S0")
                S1 = psc.tile([1, TCH], F32, tag="sc", name="S1")
                fin2 = ln_alloc_fin()
                ln_stats_bi(tsl, S0, S1, 0)
                ln_finish_half(S0, S1, *fin2, slice(0, T))
                attn_rdbmul(o_t, rden1, 1)
                proj_bi(wt, o_t, tsl, 1)
                mark("ln")
                ln_stats_bi(tsl, S0, S1, 1)
                ln_finish_half(S0, S1, *fin2, slice(T, TCH))
                ab2 = (fin2[2], fin2[3])
                last = (l == L - 1 and ti == NTC - 1)
                a2, b2 = ln_bcast(*ab2, "ln2")
                if not last:
                    ntsl = slice(((ti + 1) % NTC) * TCH,
                                 (((ti + 1) % NTC) + 1) * TCH)
                    nS0, nS1 = ln_stats(ntsl, pool=pmm)
                h2 = ln_apply(tsl, a2, b2, "g")
                if not last:
                    nab = ln_finish(nS0, nS1)
                mark("ffn1")
                ffn1 = ffn1_emit(wt, h2)
                if not last:
                    a1, b1 = ln_bcast(*nab, "ln1", evac=True)
                    h1, h8 = ln_apply(ntsl, a1, b1, "h")
                # 12. FFN2 + residual
                mark("ffn2")
                ffn2_emit(wt, ffn1, tsl)
            if l + 1 < L:
                wt = wt_next

        mark("lmhead")
        # ---------------- lm head (fp32r) ----------------
        for tt in range(NT // P):
            ps = pmm.tile([P, V], F32, tag="mm", name="mmps")
            for cc in range(NCC):
                nc.tensor.matmul(ps, x_sb[cc][:, tt * P:(tt + 1) * P],
                                 wlm_sb[cc][:, :], start=(cc == 0),
                                 stop=(cc == NCC - 1))
            lo = plog.tile([P, V], F32, tag="lg", name="lo")
            nc.vector.tensor_add(lo, ps, blm_bc)
            nc.sync.dma_start(out=out_d[tt * P:(tt + 1) * P, :], in_=lo)

    if not nc.is_finalized():
        nc.finalize()
    return nc


def prep_inputs(idx, tok_emb, pos_emb, Wq, Wk, Wv, Wo, bo, ln1_g, ln1_b,
                ln2_g, ln2_b, W1, b1, W2, b2, Wlm, blm):
    """host-side: fold LN affines into weights, build per-core input maps"""
    f32 = np.float32
    idx = np.asarray(idx)
    tok_emb = np.asarray(tok_emb, f32)
    pos_emb = np.asarray(pos_emb, f32)
    scale = C ** -0.5

    wq = np.empty((L, C, C), f32)
    wk = np.empty((L, C, C), f32)
    wv = np.empty((L, C, C), f32)
    wo = np.empty((L, C, C), f32)
    w1 = np.empty((L, C, FF), f32)
    w2 = np.empty((L, FF, C), f32)
    bq = np.empty((L, C), f32)
    bk = np.empty((L, C), f32)
    bv = np.empty((L, C), f32)
    b1f = np.empty((L, FF), f32)
    for l in range(L):
        wq_c = np.asarray(Wq[l], f32).transpose(1, 0, 2).reshape(C, C)
        wk_c = np.asarray(Wk[l], f32).transpose(1, 0, 2).reshape(C, C)
        wv_c = np.asarray(Wv[l], f32).transpose(1, 0, 2).reshape(C, C)
        g1 = np.asarray(ln1_g[l], f32)[:, None]
        b1_ = np.asarray(ln1_b[l], f32)
        g2 = np.asarray(ln2_g[l], f32)[:, None]
        b2_ = np.asarray(ln2_b[l], f32)
        wq[l] = g1 * wq_c * scale
        bq[l] = (b1_ @ wq_c) * scale
        wk[l] = g1 * wk_c
        bk[l] = b1_ @ wk_c
        wv[l] = g1 * wv_c
        bv[l] = b1_ @ wv_c
        wo[l] = np.asarray(Wo[l], f32)
        w1[l] = g2 * np.asarray(W1[l], f32)
        b1f[l] = np.asarray(b1[l], f32) + b2_ @ np.asarray(W1[l], f32)
        w2[l] = np.asarray(W2[l], f32)

    bo = np.asarray(bo, f32)
    b2a = np.asarray(b2, f32)

    # multiplicative in-block causal mask: keep where s_local <= t_local
    sp = np.arange(P)
    tri01 = (sp[:, None] <= sp[None, :]).astype(np.float16)

    pos2 = np.concatenate([pos_emb.T, pos_emb.T], axis=1)  # [C, 512]

    flags = {
        "zero_attn_bias": not (np.any(bq) or np.any(bk) or np.any(bv)
                               or np.any(bo)),
        "zero_mlp_bias": not (np.any(b1f) or np.any(b2a)),
    }

    e8sel = np.zeros((NCC, H, P), f32)
    for hq in range(NCC):
        for p_ in range(P):
            e8sel[hq, 2 * hq + p_ // HD, p_] = 1.0

    import ml_dtypes
    f8 = ml_dtypes.float8_e4m3fn

    def pack8(w, s):
        assert np.abs(w).max() * s < 440.0, "fp8 prescale overflow"
        pk = (w * s).reshape(L, 2, 2, P, C).transpose(0, 1, 3, 2, 4)
        return np.ascontiguousarray(pk).astype(f8)

    shared = {
        "e8sel": e8sel,
        "tok_emb16": tok_emb.astype(np.float16),
        "pos2T": np.ascontiguousarray(pos2).astype(np.float16),
        "tri01": np.ascontiguousarray(tri01),
        "wq8": pack8(wq, QS8), "wk8": pack8(wk, KS8),
        "wv": wv.astype(np.float16), "wo": wo.astype(np.float16),
        "w1": w1.astype(np.float16), "w2": w2.astype(np.float16),
        "bq": bq, "bk": bk, "bv": bv,
        "bo": bo, "b1": b1f, "b2": b2a,
        "wlm": np.asarray(Wlm, f32), "blm": np.asarray(blm, f32),
    }
    in_maps = []
    vocab = np.arange(V)
    for core in range(NCORES):
        toks = np.asarray(idx[core * BL:(core + 1) * BL]).reshape(-1)
        oh = (vocab[:, None] == toks[None, :]).astype(np.float16)
        m = dict(shared)
        m["onehotT"] = np.ascontiguousarray(oh)
        in_maps.append(m)
    return in_maps, flags


_NC_CACHE = {}


def get_nc(flags=None):
    if flags is None:
        flags = {"zero_attn_bias": False, "zero_mlp_bias": False}
    key = (flags["zero_attn_bias"], flags["zero_mlp_bias"])
    if key not in _NC_CACHE:
        _NC_CACHE[key] = build_bass(**flags)
    return _NC_CACHE[key]


def run(in_maps, flags=None, trace=False, **kw):
    from concourse.bass_utils import run_bass_kernel_spmd
    nc = get_nc(flags)
    return run_bass_kernel_spmd(nc, in_maps, list(range(NCORES)),
                                trace=trace, **kw)


def kernel(**inputs):
    in_maps, flags = prep_inputs(**inputs)
    res = run(in_maps, flags)
    outs = [res.results[i]["out"].reshape(BL, T, V) for i in range(NCORES)]
    return np.concatenate(outs, axis=0).astype(np.float32)
